# revision 8
# baseline (speedup 1.0000x reference)
"""BitLinear (ternary-quantized linear) Trainium2 kernel.

Computes: out = x @ ternary_quantize(weight).T
  where ternary_quantize(w) = round(clip(w / scale, -1, 1)) * scale,
        scale = max(mean(|w|), 1e-8)

Sharding: column-parallel across 8 NeuronCores — weight is sharded along
out_features (2048 per core), x is replicated, outputs concatenated.

Device kernel per core:
  - loads its fp32 weight shard, quantizes it on-device to ternary bf16
    (exact: t = (w*inv_scale > 0.5) - (w*inv_scale < -0.5), which matches
    round(clip(...)) including round-half-even at +-0.5), keeps it
    resident in SBUF,
  - streams x (pre-transposed to [K, T] bf16 on host) in token groups,
  - accumulates x_tile.T @ w_tile in PSUM over the K dimension,
  - applies `scale` during the PSUM->SBUF eviction, then DMAs out.

The scalar `scale` is computed on the host (a single reduction over the
weight); it is bit-identical to jnp's fp32 mean for this computation when
accumulated in fp64 and rounded to fp32.
"""

import os

import numpy as np
import ml_dtypes

import concourse.bass as bass
import concourse.tile as tile
from concourse import bacc, mybir
from concourse.bass_utils import run_bass_kernel_spmd

N_CORES = 8
T = 8192  # tokens
K = 4096  # in_features
O = 16384  # out_features
OS = O // N_CORES  # out_features per core (2048)
P = 128  # partitions
KT = K // P  # 32 k-tiles
NMM = 512  # moving free dim per matmul
NT = OS // NMM  # 4 n-slices per psum tile
G = 256  # tokens per group
NG = T // G  # 32 groups
MPG = G // P  # m-tiles (of 128 tokens) per group

F32 = mybir.dt.float32
BF16 = mybir.dt.bfloat16

LAST_RESULTS = None  # BassKernelResults of the most recent run (for test harness)


def _build_program(inv_scale: float, scale: float):
    nc = bacc.Bacc(
        "TRN2",
        target_bir_lowering=False,
        debug=False,
        enable_asserts=False,
        num_devices=N_CORES,
    )
    xt_d = nc.dram_tensor("xt", [K, T], BF16, kind="ExternalInput").ap()
    wt_d = nc.dram_tensor("wt", [K, OS], F32, kind="ExternalInput").ap()
    out_d = nc.dram_tensor("out", [T, OS], F32, kind="ExternalOutput").ap()

    mul = mybir.AluOpType.mult
    mn = mybir.AluOpType.min
    mx = mybir.AluOpType.max
    I8 = mybir.dt.int8

    with tile.TileContext(nc) as tc:
        with (
            tc.tile_pool(name="wq", bufs=1) as wq_pool,
            tc.tile_pool(name="wstage", bufs=3) as ws_pool,
            tc.tile_pool(name="q8t", bufs=2) as q8_pool,
            tc.tile_pool(name="xin", bufs=36) as x_pool,
            tc.tile_pool(name="osb", bufs=2) as o_pool,
            tc.tile_pool(name="acc", bufs=2, space="PSUM") as p_pool,
        ):
            # ---- Phase 0: load + quantize weight shard, keep resident ----
            # q8 = int8(w * inv_scale)   (f32->int8 convert rounds half-even,
            #                             == round(w/scale) for this data)
            # q  = bf16(clamp(q8, -1, 1)) == round(clip(w/scale, -1, 1))
            # The 33.5MB fp32 weight stream is the startup critical path
            # (~94us at HBM rate); group 0's x loads and matmuls are
            # interleaved per k-tile so the PE tracks the arriving stream
            # instead of idling behind it.
            wq = []
            xg0 = []
            psums0 = [
                p_pool.tile([P, OS], F32, tag="acc", name=f"psum_w{mi}")
                for mi in range(MPG)
            ]
            for k in range(KT):
                xt0 = x_pool.tile([P, G], BF16, tag="xin", name=f"x0_{k}")
                nc.sync.dma_start(xt0[:], xt_d[k * P : (k + 1) * P, 0:G])
                xg0.append(xt0)
                stage = ws_pool.tile([P, OS], F32, tag="wstage")
                q8 = q8_pool.tile([P, OS], I8, tag="q8t")
                q = wq_pool.tile([P, OS], BF16, tag=f"wq{k}")
                nc.sync.dma_start(stage[:], wt_d[k * P : (k + 1) * P, :])
                nc.vector.tensor_scalar(q8[:], stage[:], inv_scale, None, mul)
                nc.vector.tensor_scalar(q[:], q8[:], 1.0, -1.0, mn, mx)
                wq.append(q)
                # warmup matmuls for group 0, k-outer: both m-tiles consume
                # this k-tile as soon as it is quantized
                for mi in range(MPG):
                    lhsT = xt0[:, mi * P : (mi + 1) * P]
                    for n in range(NT):
                        nc.tensor.matmul(
                            psums0[mi][:, n * NMM : (n + 1) * NMM],
                            lhsT,
                            q[:, n * NMM : (n + 1) * NMM],
                            start=(k == 0),
                            stop=(k == KT - 1),
                        )

            # ---- Phase 1: stream x, matmul, scale on eviction ----
            def evict(psum, g, mi):
                osb = o_pool.tile([P, OS], F32, tag="osb")
                # out = psum * scale (fp32), evict PSUM -> SBUF
                nc.vector.tensor_scalar_mul(osb[:], psum[:], scale)
                t0 = g * G + mi * P
                nc.sync.dma_start(out_d[t0 : t0 + P, :], osb[:])

            for mi in range(MPG):
                evict(psums0[mi], 0, mi)

            for g in range(1, NG):
                xg = []
                for k in range(KT):
                    xt = x_pool.tile([P, G], BF16, tag="xin")
                    nc.sync.dma_start(
                        xt[:], xt_d[k * P : (k + 1) * P, g * G : (g + 1) * G]
                    )
                    xg.append(xt)
                for mi in range(MPG):
                    psum = p_pool.tile([P, OS], F32, tag="acc")
                    for k in range(KT):
                        lhsT = xg[k][:, mi * P : (mi + 1) * P]
                        for n in range(NT):
                            nc.tensor.matmul(
                                psum[:, n * NMM : (n + 1) * NMM],
                                lhsT,
                                wq[k][:, n * NMM : (n + 1) * NMM],
                                start=(k == 0),
                                stop=(k == KT - 1),
                            )
                    evict(psum, g, mi)
    nc.compile()
    return nc


def kernel(x: np.ndarray, weight: np.ndarray) -> np.ndarray:
    global LAST_RESULTS
    x = np.asarray(x, dtype=np.float32)
    w = np.asarray(weight, dtype=np.float32)
    assert x.shape == (T, K) and w.shape == (O, K)

    # scale = max(mean(|w|), 1e-8) in fp32 (fp64 accumulation rounds to the
    # same fp32 value jnp produces for this reduction)
    scale = np.float32(max(np.mean(np.abs(w), dtype=np.float64), 1e-8))
    inv_scale = np.float32(1.0) / scale

    # host-side layout prep: x transposed to [K, T] bf16; weight transposed
    # to [K, O] fp32 and sharded along out_features
    xt = np.ascontiguousarray(x.T).astype(ml_dtypes.bfloat16)
    wt = np.ascontiguousarray(w.T)  # [K, O] f32

    nc = _build_program(float(inv_scale), float(scale))

    in_maps = [
        {"xt": xt, "wt": np.ascontiguousarray(wt[:, c * OS : (c + 1) * OS])}
        for c in range(N_CORES)
    ]
    trace = bool(os.environ.get("KERNEL_TRACE"))
    LAST_RESULTS = run_bass_kernel_spmd(
        nc, in_maps, list(range(N_CORES)), trace=trace
    )
    out = np.concatenate(
        [LAST_RESULTS.results[c]["out"] for c in range(N_CORES)], axis=1
    )
    assert out.shape == (T, O) and out.dtype == np.float32
    return out


# revision 10
# speedup vs baseline: 1.0112x; 1.0112x over previous
"""BitLinear (ternary-quantized linear) Trainium2 kernel.

Computes: out = x @ ternary_quantize(weight).T
  where ternary_quantize(w) = round(clip(w / scale, -1, 1)) * scale,
        scale = max(mean(|w|), 1e-8)

Sharding: column-parallel across 8 NeuronCores — weight is sharded along
out_features (2048 per core), x is replicated, outputs concatenated.

Device kernel per core:
  - loads its fp32 weight shard, quantizes it on-device to ternary bf16
    (exact: t = (w*inv_scale > 0.5) - (w*inv_scale < -0.5), which matches
    round(clip(...)) including round-half-even at +-0.5), keeps it
    resident in SBUF,
  - streams x (pre-transposed to [K, T] bf16 on host) in token groups,
  - accumulates x_tile.T @ w_tile in PSUM over the K dimension,
  - applies `scale` during the PSUM->SBUF eviction, then DMAs out.

The scalar `scale` is computed on the host (a single reduction over the
weight); it is bit-identical to jnp's fp32 mean for this computation when
accumulated in fp64 and rounded to fp32.
"""

import os

import numpy as np
import ml_dtypes

import concourse.bass as bass
import concourse.tile as tile
from concourse import bacc, mybir
from concourse.bass_utils import run_bass_kernel_spmd

N_CORES = 8
T = 8192  # tokens
K = 4096  # in_features
O = 16384  # out_features
OS = O // N_CORES  # out_features per core (2048)
P = 128  # partitions
KT = K // P  # 32 k-tiles
NMM = 512  # moving free dim per matmul
NT = OS // NMM  # 4 n-slices per psum tile
G = 256  # tokens per group
NG = T // G  # 32 groups
MPG = G // P  # m-tiles (of 128 tokens) per group

F32 = mybir.dt.float32
BF16 = mybir.dt.bfloat16

LAST_RESULTS = None  # BassKernelResults of the most recent run (for test harness)


def _build_program(inv_scale: float, scale: float):
    nc = bacc.Bacc(
        "TRN2",
        target_bir_lowering=False,
        debug=False,
        enable_asserts=False,
        num_devices=N_CORES,
    )
    xt_d = nc.dram_tensor("xt", [K, T], BF16, kind="ExternalInput").ap()
    wt_d = nc.dram_tensor("wt", [K, OS], F32, kind="ExternalInput").ap()
    out_d = nc.dram_tensor("out", [T, OS], F32, kind="ExternalOutput").ap()

    mul = mybir.AluOpType.mult
    mn = mybir.AluOpType.min
    mx = mybir.AluOpType.max
    add = mybir.AluOpType.add
    I8 = mybir.dt.int8
    F8 = mybir.dt.float8e4  # ternary {-1,0,1} is exact in e4m3

    WD = 8  # k-tile depth of one warmup round
    WR = KT // WD  # 4 rounds
    WG = 2  # groups consumed by the warmup (m-tiles 0..3)

    with tile.TileContext(nc) as tc:
        with (
            tc.tile_pool(name="wq", bufs=1) as wq_pool,
            tc.tile_pool(name="wstage", bufs=3) as ws_pool,
            tc.tile_pool(name="q8t", bufs=2) as q8_pool,
            tc.tile_pool(name="xin", bufs=68) as x_pool,
            tc.tile_pool(name="part", bufs=1) as part_pool,
            tc.tile_pool(name="osb", bufs=2) as o_pool,
            tc.tile_pool(name="acc", bufs=2, space="PSUM") as p_pool,
        ):
            # ---- Phase 0: stream + quantize weight shard, keep resident ----
            # q8 = int8(w * inv_scale)   (f32->int8 convert rounds half-even,
            #                             == round(w/scale) for this data)
            # q  = fp8(clamp(q8, -1, 1)) == round(clip(w/scale, -1, 1)),
            #      exact in e4m3; the PE multiplies bf16 x against fp8
            #      ternary weights exactly.
            wq = []
            xw = [[], []]  # x tiles for warmup groups 0 and 1, per k
            for k in range(KT):
                for g in range(WG):
                    xt0 = x_pool.tile([P, G], BF16, tag="xin", name=f"xw{g}_{k}")
                    nc.sync.dma_start(
                        xt0[:], xt_d[k * P : (k + 1) * P, g * G : (g + 1) * G]
                    )
                    xw[g].append(xt0)
                stage = ws_pool.tile([P, OS], F32, tag="wstage")
                q8 = q8_pool.tile([P, OS], I8, tag="q8t")
                q = wq_pool.tile([P, OS], F8, tag=f"wq{k}")
                nc.sync.dma_start(stage[:], wt_d[k * P : (k + 1) * P, :])
                nc.vector.tensor_scalar(q8[:], stage[:], inv_scale, None, mul)
                nc.vector.tensor_scalar(q[:], q8[:], 1.0, -1.0, mn, mx)
                wq.append(q)

            # ---- Warmup: groups 0-1 in k-depth-8 rounds with f32 partial
            # accumulators in SBUF. The 33.5MB weight stream takes ~94us at
            # HBM rate and PSUM can only ride ~1.7us of matmul work per
            # arriving k-tile; splitting K lets later rounds backfill with
            # already-resident k-tiles so the PE stays saturated after the
            # first round.
            parts = [
                part_pool.tile([P, OS], F32, tag=f"part{wm}", name=f"part{wm}")
                for wm in range(WG * MPG)
            ]
            for r in range(WR):
                for pair in range(WG):  # (m0,m1) then (m2,m3)
                    g = pair
                    psums = [
                        p_pool.tile([P, OS], F32, tag="acc", name=f"ps_w{r}_{pair}{mi}")
                        for mi in range(MPG)
                    ]
                    for k in range(r * WD, (r + 1) * WD):
                        for mi in range(MPG):
                            lhsT = xw[g][k][:, mi * P : (mi + 1) * P]
                            for n in range(NT):
                                nc.tensor.matmul(
                                    psums[mi][:, n * NMM : (n + 1) * NMM],
                                    lhsT,
                                    wq[k][:, n * NMM : (n + 1) * NMM],
                                    start=(k == r * WD),
                                    stop=(k == (r + 1) * WD - 1),
                                )
                    for mi in range(MPG):
                        wm = pair * MPG + mi
                        if r == 0:
                            # part = psum * scale
                            nc.vector.tensor_scalar_mul(
                                parts[wm][:], psums[mi][:], scale
                            )
                        elif r < WR - 1:
                            # part += psum * scale
                            nc.vector.scalar_tensor_tensor(
                                parts[wm][:], psums[mi][:], scale, parts[wm][:],
                                op0=mul, op1=add,
                            )
                        else:
                            # final round: osb = psum * scale + part, then out
                            osb = o_pool.tile([P, OS], F32, tag="osb")
                            nc.vector.scalar_tensor_tensor(
                                osb[:], psums[mi][:], scale, parts[wm][:],
                                op0=mul, op1=add,
                            )
                            t0 = g * G + mi * P
                            nc.sync.dma_start(out_d[t0 : t0 + P, :], osb[:])

            # ---- Phase 1: stream x, matmul, scale on eviction ----
            def evict(psum, g, mi):
                osb = o_pool.tile([P, OS], F32, tag="osb")
                # out = psum * scale (fp32), evict PSUM -> SBUF
                nc.vector.tensor_scalar_mul(osb[:], psum[:], scale)
                t0 = g * G + mi * P
                nc.sync.dma_start(out_d[t0 : t0 + P, :], osb[:])

            for g in range(WG, NG):
                xg = []
                for k in range(KT):
                    xt = x_pool.tile([P, G], BF16, tag="xin")
                    nc.sync.dma_start(
                        xt[:], xt_d[k * P : (k + 1) * P, g * G : (g + 1) * G]
                    )
                    xg.append(xt)
                for mi in range(MPG):
                    psum = p_pool.tile([P, OS], F32, tag="acc")
                    for k in range(KT):
                        lhsT = xg[k][:, mi * P : (mi + 1) * P]
                        for n in range(NT):
                            nc.tensor.matmul(
                                psum[:, n * NMM : (n + 1) * NMM],
                                lhsT,
                                wq[k][:, n * NMM : (n + 1) * NMM],
                                start=(k == 0),
                                stop=(k == KT - 1),
                            )
                    evict(psum, g, mi)
    nc.compile()
    return nc


def kernel(x: np.ndarray, weight: np.ndarray) -> np.ndarray:
    global LAST_RESULTS
    x = np.asarray(x, dtype=np.float32)
    w = np.asarray(weight, dtype=np.float32)
    assert x.shape == (T, K) and w.shape == (O, K)

    # scale = max(mean(|w|), 1e-8) in fp32 (fp64 accumulation rounds to the
    # same fp32 value jnp produces for this reduction)
    scale = np.float32(max(np.mean(np.abs(w), dtype=np.float64), 1e-8))
    inv_scale = np.float32(1.0) / scale

    # host-side layout prep: x transposed to [K, T] bf16; weight transposed
    # to [K, O] fp32 and sharded along out_features
    xt = np.ascontiguousarray(x.T).astype(ml_dtypes.bfloat16)
    wt = np.ascontiguousarray(w.T)  # [K, O] f32

    nc = _build_program(float(inv_scale), float(scale))

    in_maps = [
        {"xt": xt, "wt": np.ascontiguousarray(wt[:, c * OS : (c + 1) * OS])}
        for c in range(N_CORES)
    ]
    trace = bool(os.environ.get("KERNEL_TRACE"))
    LAST_RESULTS = run_bass_kernel_spmd(
        nc, in_maps, list(range(N_CORES)), trace=trace
    )
    out = np.concatenate(
        [LAST_RESULTS.results[c]["out"] for c in range(N_CORES)], axis=1
    )
    assert out.shape == (T, O) and out.dtype == np.float32
    return out


# revision 18
# speedup vs baseline: 1.0148x; 1.0035x over previous
"""BitLinear (ternary-quantized linear) Trainium2 kernel.

Computes: out = x @ ternary_quantize(weight).T
  where ternary_quantize(w) = round(clip(w / scale, -1, 1)) * scale,
        scale = max(mean(|w|), 1e-8)

Sharding: column-parallel across 8 NeuronCores — weight is sharded along
out_features (2048 per core), x is replicated, outputs concatenated.

Device kernel per core:
  - loads its fp32 weight shard, quantizes it on-device to ternary bf16
    (exact: t = (w*inv_scale > 0.5) - (w*inv_scale < -0.5), which matches
    round(clip(...)) including round-half-even at +-0.5), keeps it
    resident in SBUF,
  - streams x (pre-transposed to [K, T] bf16 on host) in token groups,
  - accumulates x_tile.T @ w_tile in PSUM over the K dimension,
  - applies `scale` during the PSUM->SBUF eviction, then DMAs out.

The scalar `scale` is computed on the host (a single reduction over the
weight); it is bit-identical to jnp's fp32 mean for this computation when
accumulated in fp64 and rounded to fp32.
"""

import os

import numpy as np
import ml_dtypes

import concourse.bass as bass
import concourse.tile as tile
from concourse import bacc, mybir
from concourse.bass_utils import run_bass_kernel_spmd

N_CORES = 8
T = 8192  # tokens
K = 4096  # in_features
O = 16384  # out_features
OS = O // N_CORES  # out_features per core (2048)
P = 128  # partitions
KT = K // P  # 32 k-tiles
NMM = 512  # moving free dim per matmul
NT = OS // NMM  # 4 n-slices per psum tile
G = 256  # tokens per group
NG = T // G  # 32 groups
MPG = G // P  # m-tiles (of 128 tokens) per group

F32 = mybir.dt.float32
BF16 = mybir.dt.bfloat16

LAST_RESULTS = None  # BassKernelResults of the most recent run (for test harness)


def _build_program(inv_scale: float, scale: float):
    nc = bacc.Bacc(
        "TRN2",
        target_bir_lowering=False,
        debug=False,
        enable_asserts=False,
        num_devices=N_CORES,
    )
    xt_d = nc.dram_tensor("xt", [K, T], BF16, kind="ExternalInput").ap()
    wt_d = nc.dram_tensor("wt", [K, OS], F32, kind="ExternalInput").ap()
    out_d = nc.dram_tensor("out", [T, OS], F32, kind="ExternalOutput").ap()

    mul = mybir.AluOpType.mult
    mn = mybir.AluOpType.min
    mx = mybir.AluOpType.max
    add = mybir.AluOpType.add
    I8 = mybir.dt.int8
    F8 = mybir.dt.float8e4  # ternary {-1,0,1} is exact in e4m3

    WD = 8  # k-tile depth of one warmup round
    WR = KT // WD  # 4 rounds
    WG = 2  # groups consumed by the warmup (m-tiles 0..3)

    with tile.TileContext(nc) as tc:
        with (
            tc.tile_pool(name="wq", bufs=1) as wq_pool,
            tc.tile_pool(name="wstage", bufs=3) as ws_pool,
            tc.tile_pool(name="q8t", bufs=2) as q8_pool,
            tc.tile_pool(name="xin", bufs=68) as x_pool,
            tc.tile_pool(name="part", bufs=1) as part_pool,
            tc.tile_pool(name="osb", bufs=2) as o_pool,
            tc.tile_pool(name="acc", bufs=4, space="PSUM") as p_pool,
        ):
            # ---- Phase 0: stream + quantize weight shard, keep resident ----
            # q8 = int8(w * inv_scale)   (f32->int8 convert rounds half-even,
            #                             == round(w/scale) for this data)
            # q  = fp8(clamp(q8, -1, 1)) == round(clip(w/scale, -1, 1)),
            #      exact in e4m3; the PE multiplies bf16 x against fp8
            #      ternary weights exactly.
            wq = []
            xw = [[], []]  # x tiles for warmup groups 0 and 1, per k
            for k in range(KT):
                for g in range(WG):
                    xt0 = x_pool.tile([P, G], BF16, tag="xin", name=f"xw{g}_{k}")
                    nc.sync.dma_start(
                        xt0[:], xt_d[k * P : (k + 1) * P, g * G : (g + 1) * G]
                    )
                    xw[g].append(xt0)
                stage = ws_pool.tile([P, OS], F32, tag="wstage")
                q8 = q8_pool.tile([P, OS], I8, tag="q8t")
                q = wq_pool.tile([P, OS], F8, tag=f"wq{k}")
                nc.sync.dma_start(stage[:], wt_d[k * P : (k + 1) * P, :])
                nc.vector.tensor_scalar(q8[:], stage[:], inv_scale, None, mul)
                nc.vector.tensor_scalar(q[:], q8[:], 1.0, -1.0, mn, mx)
                wq.append(q)

            # ---- Warmup: groups 0-1 in k-depth-8 rounds with f32 partial
            # accumulators in SBUF. The 33.5MB weight stream takes ~94us at
            # HBM rate and PSUM can only ride ~1.7us of matmul work per
            # arriving k-tile; splitting K lets later rounds backfill with
            # already-resident k-tiles so the PE stays saturated after the
            # first round. All 4 warm m-tiles stay live on half-width (2-bank)
            # PSUM accumulators so each merge overlaps the other m-tiles'
            # matmuls (full-width pairs would stall ~1.6us at every round
            # seam waiting on the eviction).
            HOS = OS // 2  # psum accumulator width (2 banks)
            NH = NT // 2  # 512-wide matmuls per half
            parts = [
                part_pool.tile([P, OS], F32, tag=f"part{wm}", name=f"part{wm}")
                for wm in range(WG * MPG)
            ]
            kranges = [(r * WD, (r + 1) * WD) for r in range(WR)]
            for r, (ka, kb) in enumerate(kranges):
                last_r = r == len(kranges) - 1
                for h in range(2):
                    hs = slice(h * HOS, (h + 1) * HOS)
                    psums = [
                        p_pool.tile([P, HOS], F32, tag="acc", name=f"ps_w{r}{h}{wm}")
                        for wm in range(WG * MPG)
                    ]
                    for k in range(ka, kb):
                        for wm in range(WG * MPG):
                            g, mi = wm // MPG, wm % MPG
                            lhsT = xw[g][k][:, mi * P : (mi + 1) * P]
                            for n in range(NH):
                                nc.tensor.matmul(
                                    psums[wm][:, n * NMM : (n + 1) * NMM],
                                    lhsT,
                                    wq[k][:, h * HOS + n * NMM : h * HOS + (n + 1) * NMM],
                                    start=(k == ka),
                                    stop=(k == kb - 1),
                                )
                    for wm in range(WG * MPG):
                        if r == 0:
                            # part = psum * scale
                            nc.vector.tensor_scalar_mul(
                                parts[wm][:, hs], psums[wm][:], scale
                            )
                        else:
                            # part += psum * scale (final round included: the
                            # completed f32 partial IS the output tile)
                            nc.vector.scalar_tensor_tensor(
                                parts[wm][:, hs], psums[wm][:], scale,
                                parts[wm][:, hs], op0=mul, op1=add,
                            )
                        if last_r and h == 1:
                            g, mi = wm // MPG, wm % MPG
                            t0 = g * G + mi * P
                            nc.sync.dma_start(out_d[t0 : t0 + P, :], parts[wm][:])

            # ---- Phase 1: stream x, matmul, scale on eviction ----
            for g in range(WG, NG):
                xg = []
                for k in range(KT):
                    xt = x_pool.tile([P, G], BF16, tag="xin")
                    nc.sync.dma_start(
                        xt[:], xt_d[k * P : (k + 1) * P, g * G : (g + 1) * G]
                    )
                    xg.append(xt)
                for mi in range(MPG):
                    # two half-width accumulators per m-tile (same 4 columns
                    # of PSUM as a full-width tile; shares slots with warmup)
                    ph = [
                        p_pool.tile([P, HOS], F32, tag="acc", name=f"ph{h}")
                        for h in range(2)
                    ]
                    for k in range(KT):
                        lhsT = xg[k][:, mi * P : (mi + 1) * P]
                        for h in range(2):
                            for n in range(NH):
                                nc.tensor.matmul(
                                    ph[h][:, n * NMM : (n + 1) * NMM],
                                    lhsT,
                                    wq[k][:, h * HOS + n * NMM : h * HOS + (n + 1) * NMM],
                                    start=(k == 0),
                                    stop=(k == KT - 1),
                                )
                    osb = o_pool.tile([P, OS], F32, tag="osb")
                    for h in range(2):
                        nc.vector.tensor_scalar_mul(
                            osb[:, h * HOS : (h + 1) * HOS], ph[h][:], scale
                        )
                    t0 = g * G + mi * P
                    nc.sync.dma_start(out_d[t0 : t0 + P, :], osb[:])
    nc.compile()
    return nc


def kernel(x: np.ndarray, weight: np.ndarray) -> np.ndarray:
    global LAST_RESULTS
    x = np.asarray(x, dtype=np.float32)
    w = np.asarray(weight, dtype=np.float32)
    assert x.shape == (T, K) and w.shape == (O, K)

    # scale = max(mean(|w|), 1e-8) in fp32 (fp64 accumulation rounds to the
    # same fp32 value jnp produces for this reduction)
    scale = np.float32(max(np.mean(np.abs(w), dtype=np.float64), 1e-8))
    inv_scale = np.float32(1.0) / scale

    # host-side layout prep: x transposed to [K, T] bf16; weight transposed
    # to [K, O] fp32 and sharded along out_features
    xt = np.ascontiguousarray(x.T).astype(ml_dtypes.bfloat16)
    wt = np.ascontiguousarray(w.T)  # [K, O] f32

    nc = _build_program(float(inv_scale), float(scale))

    in_maps = [
        {"xt": xt, "wt": np.ascontiguousarray(wt[:, c * OS : (c + 1) * OS])}
        for c in range(N_CORES)
    ]
    trace = bool(os.environ.get("KERNEL_TRACE"))
    LAST_RESULTS = run_bass_kernel_spmd(
        nc, in_maps, list(range(N_CORES)), trace=trace
    )
    out = np.concatenate(
        [LAST_RESULTS.results[c]["out"] for c in range(N_CORES)], axis=1
    )
    assert out.shape == (T, O) and out.dtype == np.float32
    return out


# revision 21
# speedup vs baseline: 1.0153x; 1.0006x over previous
"""BitLinear (ternary-quantized linear) Trainium2 kernel.

Computes: out = x @ ternary_quantize(weight).T
  where ternary_quantize(w) = round(clip(w / scale, -1, 1)) * scale,
        scale = max(mean(|w|), 1e-8)

Sharding: column-parallel across 8 NeuronCores — weight is sharded along
out_features (2048 per core), x is replicated, outputs concatenated.

Device kernel per core:
  - streams its fp32 weight shard, quantizes it on-device to exact ternary
    fp8e4 (int8-convert rounds half-even == round(clip(w/scale,-1,1))),
    keeps it resident in SBUF,
  - streams x (pre-transposed to [K, T] bf16 on host) in token groups and
    accumulates x_tile.T @ w_tile in PSUM over K (the PE's bf16 x fp8
    multiply is exact for ternary weights),
  - overlaps the ~94us weight stream with groups 0-1 via k-split rounds
    into f32 partial accumulators on half-width PSUM tiles,
  - applies `scale` during the PSUM->SBUF eviction, then DMAs out.

The scalar `scale` is computed on the host (a single reduction over the
weight); it is bit-identical to jnp's fp32 mean for this computation when
accumulated in fp64 and rounded to fp32.
"""

import os

import numpy as np
import ml_dtypes

import concourse.bass as bass
import concourse.tile as tile
from concourse import bacc, mybir
from concourse.bass_utils import run_bass_kernel_spmd

N_CORES = 8
T = 8192  # tokens
K = 4096  # in_features
O = 16384  # out_features
OS = O // N_CORES  # out_features per core (2048)
P = 128  # partitions
KT = K // P  # 32 k-tiles
NMM = 512  # moving free dim per matmul
NT = OS // NMM  # 4 n-slices per psum tile
G = 256  # tokens per group
NG = T // G  # 32 groups
MPG = G // P  # m-tiles (of 128 tokens) per group

F32 = mybir.dt.float32
BF16 = mybir.dt.bfloat16

LAST_RESULTS = None  # BassKernelResults of the most recent run (for test harness)


def _build_program(inv_scale: float, scale: float):
    nc = bacc.Bacc(
        "TRN2",
        target_bir_lowering=False,
        debug=False,
        enable_asserts=False,
        num_devices=N_CORES,
    )
    xt_d = nc.dram_tensor("xt", [K, T], BF16, kind="ExternalInput").ap()
    wt_d = nc.dram_tensor("wt", [K, OS], F32, kind="ExternalInput").ap()
    out_d = nc.dram_tensor("out", [T, OS], F32, kind="ExternalOutput").ap()

    mul = mybir.AluOpType.mult
    mn = mybir.AluOpType.min
    mx = mybir.AluOpType.max
    add = mybir.AluOpType.add
    I8 = mybir.dt.int8
    F8 = mybir.dt.float8e4  # ternary {-1,0,1} is exact in e4m3

    WD = 8  # k-tile depth of one warmup round
    WR = KT // WD  # 4 rounds
    WG = 2  # groups consumed by the warmup (m-tiles 0..3)

    with tile.TileContext(nc) as tc:
        with (
            tc.tile_pool(name="wq", bufs=1) as wq_pool,
            tc.tile_pool(name="wstage", bufs=3) as ws_pool,
            tc.tile_pool(name="q8t", bufs=2) as q8_pool,
            tc.tile_pool(name="xin", bufs=68) as x_pool,
            tc.tile_pool(name="part", bufs=1) as part_pool,
            tc.tile_pool(name="osb", bufs=2) as o_pool,
            tc.tile_pool(name="acc", bufs=4, space="PSUM") as p_pool,
        ):
            # ---- Phase 0: stream + quantize weight shard, keep resident ----
            # q8 = int8(w * inv_scale)   (f32->int8 convert rounds half-even,
            #                             == round(w/scale) for this data)
            # q  = fp8(clamp(q8, -1, 1)) == round(clip(w/scale, -1, 1)),
            #      exact in e4m3; the PE multiplies bf16 x against fp8
            #      ternary weights exactly.
            wq = []
            xw = [[], []]  # x tiles for warmup groups 0 and 1, per k
            for k in range(KT):
                for g in range(WG):
                    xt0 = x_pool.tile([P, G], BF16, tag="xin", name=f"xw{g}_{k}")
                    nc.sync.dma_start(
                        xt0[:], xt_d[k * P : (k + 1) * P, g * G : (g + 1) * G]
                    )
                    xw[g].append(xt0)
                stage = ws_pool.tile([P, OS], F32, tag="wstage")
                q8 = q8_pool.tile([P, OS], I8, tag="q8t")
                q = wq_pool.tile([P, OS], F8, tag=f"wq{k}")
                nc.sync.dma_start(stage[:], wt_d[k * P : (k + 1) * P, :])
                nc.vector.tensor_scalar(q8[:], stage[:], inv_scale, None, mul)
                nc.vector.tensor_scalar(q[:], q8[:], 1.0, -1.0, mn, mx)
                wq.append(q)

            # ---- Warmup: groups 0-1 in k-depth-8 rounds with f32 partial
            # accumulators in SBUF. The 33.5MB weight stream takes ~94us at
            # HBM rate and PSUM can only ride ~1.7us of matmul work per
            # arriving k-tile; splitting K lets later rounds backfill with
            # already-resident k-tiles so the PE stays saturated after the
            # first round. All 4 warm m-tiles stay live on half-width (2-bank)
            # PSUM accumulators so each merge overlaps the other m-tiles'
            # matmuls (full-width pairs would stall ~1.6us at every round
            # seam waiting on the eviction).
            HOS = OS // 2  # psum accumulator width (2 banks)
            NH = NT // 2  # 512-wide matmuls per half
            parts = [
                part_pool.tile([P, OS], F32, tag=f"part{wm}", name=f"part{wm}")
                for wm in range(WG * MPG)
            ]
            kranges = [(r * WD, (r + 1) * WD) for r in range(WR)]
            for r, (ka, kb) in enumerate(kranges):
                last_r = r == len(kranges) - 1
                for h in range(2):
                    hs = slice(h * HOS, (h + 1) * HOS)
                    psums = [
                        p_pool.tile([P, HOS], F32, tag="acc", name=f"ps_w{r}{h}{wm}")
                        for wm in range(WG * MPG)
                    ]
                    for k in range(ka, kb):
                        for wm in range(WG * MPG):
                            g, mi = wm // MPG, wm % MPG
                            lhsT = xw[g][k][:, mi * P : (mi + 1) * P]
                            for n in range(NH):
                                nc.tensor.matmul(
                                    psums[wm][:, n * NMM : (n + 1) * NMM],
                                    lhsT,
                                    wq[k][:, h * HOS + n * NMM : h * HOS + (n + 1) * NMM],
                                    start=(k == ka),
                                    stop=(k == kb - 1),
                                )
                    for wm in range(WG * MPG):
                        if r == 0:
                            # part = psum * scale
                            nc.vector.tensor_scalar_mul(
                                parts[wm][:, hs], psums[wm][:], scale
                            )
                        else:
                            # part += psum * scale (final round included: the
                            # completed f32 partial IS the output tile)
                            nc.vector.scalar_tensor_tensor(
                                parts[wm][:, hs], psums[wm][:], scale,
                                parts[wm][:, hs], op0=mul, op1=add,
                            )
                        if last_r and h == 1:
                            g, mi = wm // MPG, wm % MPG
                            t0 = g * G + mi * P
                            nc.sync.dma_start(out_d[t0 : t0 + P, :], parts[wm][:])

            # ---- Phase 1: stream x, matmul, scale on eviction ----
            for g in range(WG, NG):
                xg = []
                for k in range(KT):
                    xt = x_pool.tile([P, G], BF16, tag="xin")
                    nc.sync.dma_start(
                        xt[:], xt_d[k * P : (k + 1) * P, g * G : (g + 1) * G]
                    )
                    xg.append(xt)
                for mi in range(MPG):
                    # two half-width accumulators per m-tile (same 4 columns
                    # of PSUM as a full-width tile; shares slots with warmup).
                    # The very last m-tile runs h-outer so half 0's evict+DMA
                    # hides under half 1's matmuls, shortening the kernel tail.
                    last_tile = g == NG - 1 and mi == MPG - 1
                    ph = [
                        p_pool.tile([P, HOS], F32, tag="acc", name=f"ph{h}")
                        for h in range(2)
                    ]
                    osb = o_pool.tile([P, OS], F32, tag="osb")
                    t0 = g * G + mi * P

                    def emit_mm(h, k):
                        lhsT = xg[k][:, mi * P : (mi + 1) * P]
                        for n in range(NH):
                            nc.tensor.matmul(
                                ph[h][:, n * NMM : (n + 1) * NMM],
                                lhsT,
                                wq[k][:, h * HOS + n * NMM : h * HOS + (n + 1) * NMM],
                                start=(k == 0),
                                stop=(k == KT - 1),
                            )

                    def emit_out(h):
                        hs = slice(h * HOS, (h + 1) * HOS)
                        nc.vector.tensor_scalar_mul(osb[:, hs], ph[h][:], scale)
                        nc.sync.dma_start(out_d[t0 : t0 + P, hs], osb[:, hs])

                    if last_tile:
                        for h in range(2):
                            for k in range(KT):
                                emit_mm(h, k)
                            emit_out(h)
                    else:
                        for k in range(KT):
                            for h in range(2):
                                emit_mm(h, k)
                        for h in range(2):
                            emit_out(h)
    nc.compile()
    return nc


def kernel(x: np.ndarray, weight: np.ndarray) -> np.ndarray:
    global LAST_RESULTS
    x = np.asarray(x, dtype=np.float32)
    w = np.asarray(weight, dtype=np.float32)
    assert x.shape == (T, K) and w.shape == (O, K)

    # scale = max(mean(|w|), 1e-8) in fp32 (fp64 accumulation rounds to the
    # same fp32 value jnp produces for this reduction)
    scale = np.float32(max(np.mean(np.abs(w), dtype=np.float64), 1e-8))
    inv_scale = np.float32(1.0) / scale

    # host-side layout prep: x transposed to [K, T] bf16; weight transposed
    # to [K, O] fp32 and sharded along out_features
    xt = np.ascontiguousarray(x.T).astype(ml_dtypes.bfloat16)
    wt = np.ascontiguousarray(w.T)  # [K, O] f32

    nc = _build_program(float(inv_scale), float(scale))

    in_maps = [
        {"xt": xt, "wt": np.ascontiguousarray(wt[:, c * OS : (c + 1) * OS])}
        for c in range(N_CORES)
    ]
    trace = bool(os.environ.get("KERNEL_TRACE"))
    LAST_RESULTS = run_bass_kernel_spmd(
        nc, in_maps, list(range(N_CORES)), trace=trace
    )
    out = np.concatenate(
        [LAST_RESULTS.results[c]["out"] for c in range(N_CORES)], axis=1
    )
    assert out.shape == (T, O) and out.dtype == np.float32
    return out


# revision 24
# speedup vs baseline: 1.0182x; 1.0028x over previous
"""BitLinear (ternary-quantized linear) Trainium2 kernel.

Computes: out = x @ ternary_quantize(weight).T
  where ternary_quantize(w) = round(clip(w / scale, -1, 1)) * scale,
        scale = max(mean(|w|), 1e-8)

Sharding: column-parallel across 8 NeuronCores — weight is sharded along
out_features (2048 per core), x is replicated, outputs concatenated.

Device kernel per core:
  - streams its fp32 weight shard, quantizes it on-device to exact ternary
    fp8e4 (int8-convert rounds half-even == round(clip(w/scale,-1,1))),
    keeps it resident in SBUF,
  - streams x (pre-transposed to [K, T] bf16 on host) in token groups and
    accumulates x_tile.T @ w_tile in PSUM over K (the PE's bf16 x fp8
    multiply is exact for ternary weights),
  - overlaps the ~94us weight stream with groups 0-1 via k-split rounds
    into f32 partial accumulators on half-width PSUM tiles,
  - applies `scale` during the PSUM->SBUF eviction, then DMAs out.

The scalar `scale` is computed on the host (a single reduction over the
weight); it is bit-identical to jnp's fp32 mean for this computation when
accumulated in fp64 and rounded to fp32.
"""

import os

import numpy as np
import ml_dtypes

import concourse.bass as bass
import concourse.tile as tile
from concourse import bacc, mybir
from concourse.bass_utils import run_bass_kernel_spmd

N_CORES = 8
T = 8192  # tokens
K = 4096  # in_features
O = 16384  # out_features
OS = O // N_CORES  # out_features per core (2048)
P = 128  # partitions
KT = K // P  # 32 k-tiles
NMM = 512  # moving free dim per matmul
NT = OS // NMM  # 4 n-slices per psum tile
G = 512  # tokens per group (1KB x-DMA partition lines, halves descriptor count)
NG = T // G  # 16 groups
MPG = G // P  # m-tiles (of 128 tokens) per group

F32 = mybir.dt.float32
BF16 = mybir.dt.bfloat16

LAST_RESULTS = None  # BassKernelResults of the most recent run (for test harness)


def _build_program(inv_scale: float, scale: float):
    nc = bacc.Bacc(
        "TRN2",
        target_bir_lowering=False,
        debug=False,
        enable_asserts=False,
        num_devices=N_CORES,
    )
    xt_d = nc.dram_tensor("xt", [K, T], BF16, kind="ExternalInput").ap()
    wt_d = nc.dram_tensor("wt", [K, OS], F32, kind="ExternalInput").ap()
    out_d = nc.dram_tensor("out", [T, OS], F32, kind="ExternalOutput").ap()

    mul = mybir.AluOpType.mult
    mn = mybir.AluOpType.min
    mx = mybir.AluOpType.max
    add = mybir.AluOpType.add
    I8 = mybir.dt.int8
    F8 = mybir.dt.float8e4  # ternary {-1,0,1} is exact in e4m3

    WD = 8  # k-tile depth of one warmup round
    WR = KT // WD  # 4 rounds
    WG = 1  # groups consumed by the warmup (m-tiles 0..3)

    with tile.TileContext(nc) as tc:
        with (
            tc.tile_pool(name="wq", bufs=1) as wq_pool,
            tc.tile_pool(name="wstage", bufs=3) as ws_pool,
            tc.tile_pool(name="q8t", bufs=2) as q8_pool,
            tc.tile_pool(name="xin", bufs=34) as x_pool,
            tc.tile_pool(name="part", bufs=1) as part_pool,
            tc.tile_pool(name="osb", bufs=2) as o_pool,
            tc.tile_pool(name="acc", bufs=4, space="PSUM") as p_pool,
        ):
            # ---- Phase 0: stream + quantize weight shard, keep resident ----
            # q8 = int8(w * inv_scale)   (f32->int8 convert rounds half-even,
            #                             == round(w/scale) for this data)
            # q  = fp8(clamp(q8, -1, 1)) == round(clip(w/scale, -1, 1)),
            #      exact in e4m3; the PE multiplies bf16 x against fp8
            #      ternary weights exactly.
            wq = []
            xw = [[], []]  # x tiles for warmup groups 0 and 1, per k
            for k in range(KT):
                for g in range(WG):
                    xt0 = x_pool.tile([P, G], BF16, tag="xin", name=f"xw{g}_{k}")
                    nc.sync.dma_start(
                        xt0[:], xt_d[k * P : (k + 1) * P, g * G : (g + 1) * G]
                    )
                    xw[g].append(xt0)
                stage = ws_pool.tile([P, OS], F32, tag="wstage")
                q8 = q8_pool.tile([P, OS], I8, tag="q8t")
                q = wq_pool.tile([P, OS], F8, tag=f"wq{k}")
                nc.sync.dma_start(stage[:], wt_d[k * P : (k + 1) * P, :])
                nc.vector.tensor_scalar(q8[:], stage[:], inv_scale, None, mul)
                nc.vector.tensor_scalar(q[:], q8[:], 1.0, -1.0, mn, mx)
                wq.append(q)

            # ---- Warmup: groups 0-1 in k-depth-8 rounds with f32 partial
            # accumulators in SBUF. The 33.5MB weight stream takes ~94us at
            # HBM rate and PSUM can only ride ~1.7us of matmul work per
            # arriving k-tile; splitting K lets later rounds backfill with
            # already-resident k-tiles so the PE stays saturated after the
            # first round. All 4 warm m-tiles stay live on half-width (2-bank)
            # PSUM accumulators so each merge overlaps the other m-tiles'
            # matmuls (full-width pairs would stall ~1.6us at every round
            # seam waiting on the eviction).
            HOS = OS // 2  # psum accumulator width (2 banks)
            NH = NT // 2  # 512-wide matmuls per half
            parts = [
                part_pool.tile([P, OS], F32, tag=f"part{wm}", name=f"part{wm}")
                for wm in range(WG * MPG)
            ]
            kranges = [(r * WD, (r + 1) * WD) for r in range(WR)]
            for r, (ka, kb) in enumerate(kranges):
                last_r = r == len(kranges) - 1
                for h in range(2):
                    hs = slice(h * HOS, (h + 1) * HOS)
                    psums = [
                        p_pool.tile([P, HOS], F32, tag="acc", name=f"ps_w{r}{h}{wm}")
                        for wm in range(WG * MPG)
                    ]
                    for k in range(ka, kb):
                        for wm in range(WG * MPG):
                            g, mi = wm // MPG, wm % MPG
                            lhsT = xw[g][k][:, mi * P : (mi + 1) * P]
                            for n in range(NH):
                                nc.tensor.matmul(
                                    psums[wm][:, n * NMM : (n + 1) * NMM],
                                    lhsT,
                                    wq[k][:, h * HOS + n * NMM : h * HOS + (n + 1) * NMM],
                                    start=(k == ka),
                                    stop=(k == kb - 1),
                                )
                    for wm in range(WG * MPG):
                        if r == 0:
                            # part = psum * scale
                            nc.vector.tensor_scalar_mul(
                                parts[wm][:, hs], psums[wm][:], scale
                            )
                        else:
                            # part += psum * scale (final round included: the
                            # completed f32 partial IS the output tile)
                            nc.vector.scalar_tensor_tensor(
                                parts[wm][:, hs], psums[wm][:], scale,
                                parts[wm][:, hs], op0=mul, op1=add,
                            )
                        if last_r and h == 1:
                            g, mi = wm // MPG, wm % MPG
                            t0 = g * G + mi * P
                            nc.sync.dma_start(out_d[t0 : t0 + P, :], parts[wm][:])

            # ---- Phase 1: stream x, matmul, scale on eviction ----
            for g in range(WG, NG):
                xg = []
                for k in range(KT):
                    xt = x_pool.tile([P, G], BF16, tag="xin")
                    nc.sync.dma_start(
                        xt[:], xt_d[k * P : (k + 1) * P, g * G : (g + 1) * G]
                    )
                    xg.append(xt)
                for mi in range(MPG):
                    # two half-width accumulators per m-tile (same 4 columns
                    # of PSUM as a full-width tile; shares slots with warmup).
                    # The very last m-tile runs h-outer so half 0's evict+DMA
                    # hides under half 1's matmuls, shortening the kernel tail.
                    last_tile = g == NG - 1 and mi == MPG - 1
                    ph = [
                        p_pool.tile([P, HOS], F32, tag="acc", name=f"ph{h}")
                        for h in range(2)
                    ]
                    osb = o_pool.tile([P, OS], F32, tag="osb")
                    t0 = g * G + mi * P

                    def emit_mm(h, k):
                        lhsT = xg[k][:, mi * P : (mi + 1) * P]
                        for n in range(NH):
                            nc.tensor.matmul(
                                ph[h][:, n * NMM : (n + 1) * NMM],
                                lhsT,
                                wq[k][:, h * HOS + n * NMM : h * HOS + (n + 1) * NMM],
                                start=(k == 0),
                                stop=(k == KT - 1),
                            )

                    def emit_out(h):
                        hs = slice(h * HOS, (h + 1) * HOS)
                        nc.vector.tensor_scalar_mul(osb[:, hs], ph[h][:], scale)
                        nc.sync.dma_start(out_d[t0 : t0 + P, hs], osb[:, hs])

                    if last_tile:
                        for h in range(2):
                            for k in range(KT):
                                emit_mm(h, k)
                            emit_out(h)
                    else:
                        for k in range(KT):
                            for h in range(2):
                                emit_mm(h, k)
                        for h in range(2):
                            emit_out(h)
    nc.compile()
    return nc


def kernel(x: np.ndarray, weight: np.ndarray) -> np.ndarray:
    global LAST_RESULTS
    x = np.asarray(x, dtype=np.float32)
    w = np.asarray(weight, dtype=np.float32)
    assert x.shape == (T, K) and w.shape == (O, K)

    # scale = max(mean(|w|), 1e-8) in fp32 (fp64 accumulation rounds to the
    # same fp32 value jnp produces for this reduction)
    scale = np.float32(max(np.mean(np.abs(w), dtype=np.float64), 1e-8))
    inv_scale = np.float32(1.0) / scale

    # host-side layout prep: x transposed to [K, T] bf16; weight transposed
    # to [K, O] fp32 and sharded along out_features
    xt = np.ascontiguousarray(x.T).astype(ml_dtypes.bfloat16)
    wt = np.ascontiguousarray(w.T)  # [K, O] f32

    nc = _build_program(float(inv_scale), float(scale))

    in_maps = [
        {"xt": xt, "wt": np.ascontiguousarray(wt[:, c * OS : (c + 1) * OS])}
        for c in range(N_CORES)
    ]
    trace = bool(os.environ.get("KERNEL_TRACE"))
    LAST_RESULTS = run_bass_kernel_spmd(
        nc, in_maps, list(range(N_CORES)), trace=trace
    )
    out = np.concatenate(
        [LAST_RESULTS.results[c]["out"] for c in range(N_CORES)], axis=1
    )
    assert out.shape == (T, O) and out.dtype == np.float32
    return out
